# revision 1
# baseline (speedup 1.0000x reference)
"""Trainium2 Bass kernel: conv2d(3x3,VALID) + bias -> min over C_out -> tanh(tanh).

Full-input contract: kernel(**inputs) takes the unsharded inputs
  x:           [32, 16, 256, 256] f32
  conv_weight: [64, 16, 3, 3]     f32
  conv_bias:   [64]               f32
and returns [32, 1, 254, 254] f32.

Strategy (data-parallel over batch, 4 images per core on 8 cores):
The conv is cast as matmuls over a "kw-shifted slab" in SBUF:
  slab[kw*16+c, t] = x[c, t+kw]   (kw in 0..6, flattened image cols t)
plus a ones row (partition 112) that carries the bias through the matmul.
For a block of 640 flat positions p = base + 5*m + j (m in 0..127, j in 0..4):
  out[m, (j,o)] = sum_kh  slab[:, base+kh*256+5m].T @ wmov[kh]
with wmov[kh][kw*16+c, j*64+o] = W[o,c,kh,kw-j] (0 when kw-j not in 0..2).
This yields PSUM [128 positions, 5 shifts, 64 channels]; the channel-min is
then a free-dim reduce_min on DVE, followed by tanh(tanh()) on ACT.
Computed on the full 256-wide rows; the host drops the 2 garbage columns/rows.
"""

import sys
import types

import numpy as np

# ---------------------------------------------------------------------------
# NTFF profile hook registration (the container's antenv stub lacks
# axon_hooks; registering it enables trace=True for profiling runs).
def _install_axon_hooks():
    try:
        import antenv.axon_hooks  # noqa: F401
        return
    except ImportError:
        pass
    try:
        import antenv
        from trn_agent_boot.trn_boot import _ntff_profile_via_ctypes
    except ImportError:
        return
    mod = types.ModuleType("antenv.axon_hooks")
    _hook = [None]
    mod.set_axon_ntff_profile_hook = lambda h: _hook.__setitem__(0, h)
    mod.get_axon_ntff_profile_hook = lambda: _hook[0]
    sys.modules["antenv.axon_hooks"] = mod
    antenv.axon_hooks = mod
    try:
        mod.set_axon_ntff_profile_hook(
            _ntff_profile_via_ctypes("/opt/axon/libaxon_pjrt.so")
        )
    except Exception:
        pass


_install_axon_hooks()

import concourse.bass as bass  # noqa: E402
import concourse.tile as tile  # noqa: E402
from concourse import bacc, mybir  # noqa: E402
from concourse.bass_utils import run_bass_kernel_spmd  # noqa: E402

N_CORES = 8
IMGS_PER_CORE = 4
C_IN, H, W = 16, 256, 256
C_OUT = 64
OH = OW = 254

J = 5                 # position shifts per matmul column group
WK = 7                # kw taps present in the contraction (0..6)
KDIM = WK * C_IN + 1  # 113 partitions: 7 kw-shifts x 16 ch + ones row
NFREE = J * C_OUT     # 320 moving columns
BLK = 128 * J         # 640 flat positions per block
IMG = H * W           # 65536
PAD_COLS = 66560      # 260 rows of 256 (2 extra conv rows + slack)
NBLOCKS = 102         # blocks 0..101 cover flat positions 0..65279

WIN_BLOCKS = 51       # two slab windows per image
WIN_STRIDE = WIN_BLOCKS * BLK      # 32640
WIN_COLS = 33792                   # window cols; max read rel offset 33153
CHUNKS = [8, 8, 8, 8, 8, 8, 3]     # blocks per output chunk inside a window


def _prep_inputs(x, conv_weight, conv_bias):
    """Host-side packing: slab-layout fp16 image tensor and matmul weights.

    x7[i, kw*16+c, t] = x[i, c, t+kw] (kw in 0..6), row 112 = ones — exactly
    the SBUF slab layout, so device loads are single contiguous 113-partition
    DMAs (measured ~270 GB/s vs ~39 GB/s for overlapping-AP reads).
    """
    n = x.shape[0]
    xf = x.reshape(n, C_IN, IMG).astype(np.float16)
    x7 = np.zeros((n, KDIM, PAD_COLS), dtype=np.float16)
    for kw in range(WK):
        lo = max(0, IMG - kw)
        x7[:, kw * C_IN:(kw + 1) * C_IN, :lo] = xf[:, :, kw:kw + lo]
    x7[:, KDIM - 1, :] = 1.0

    # wmov[kh][kw*16+c, j*64+o] = W[o, c, kh, kw-j] for kw-j in 0..2
    wmov = np.zeros((KDIM, 3, J, C_OUT), dtype=np.float32)
    for kh in range(3):
        for kw in range(WK):
            for j in range(J):
                kk = kw - j
                if 0 <= kk <= 2:
                    # rows kw*16..kw*16+15, cols (j,o)
                    wmov[kw * C_IN:(kw + 1) * C_IN, kh, j, :] = (
                        conv_weight[:, :, kh, kk].T
                    )
    wmov[KDIM - 1, 0, :, :] = conv_bias[None, :]  # bias via ones row, kh=0 only
    # dram layout [113, 3*320] with col = kh*320 + j*64 + o
    wmov = wmov.reshape(KDIM, 3 * NFREE).astype(np.float16)
    return x7, wmov


def _build_program():
    nc = bacc.Bacc(
        "TRN2", target_bir_lowering=False, debug=False, num_devices=N_CORES
    )
    f16 = mybir.dt.float16
    f32 = mybir.dt.float32

    x_d = nc.dram_tensor(
        "x", [IMGS_PER_CORE, KDIM, PAD_COLS], f16, kind="ExternalInput"
    )
    w_d = nc.dram_tensor("w", [KDIM, 3 * NFREE], f16, kind="ExternalInput")
    y_d = nc.dram_tensor(
        "y", [IMGS_PER_CORE, NBLOCKS * BLK], f32, kind="ExternalOutput"
    )

    with tile.TileContext(nc) as tc:
        with (
            tc.tile_pool(name="wpool", bufs=1) as wpool,
            tc.tile_pool(name="slab", bufs=2) as slab_pool,
            tc.tile_pool(name="stage", bufs=4) as stage_pool,
            tc.tile_pool(name="psum", bufs=4, space="PSUM") as psum_pool,
        ):
            w_t = wpool.tile([KDIM, 3 * NFREE], f16)
            nc.sync.dma_start(w_t[:], w_d[:])

            windows = [
                (i, wi) for i in range(IMGS_PER_CORE) for wi in range(2)
            ]

            def load_window(idx):
                i, wi = windows[idx]
                wstart = wi * WIN_STRIDE
                slab = slab_pool.tile([KDIM, WIN_COLS], f16)
                # Contiguous loads from the host-replicated slab layout.
                # CRITICAL: 112-partition transfers spray across all 16 SDMA
                # engines (~210 GB/s); a 113-partition transfer falls back to
                # a single engine (~27 GB/s). Load the ones row separately.
                half = WIN_COLS // 2
                for h in range(2):
                    nc.sync.dma_start(
                        slab[0:112, h * half:(h + 1) * half],
                        x_d[i, 0:112,
                            wstart + h * half:wstart + (h + 1) * half],
                    )
                nc.sync.dma_start(
                    slab[112:113, :],
                    x_d[i, 112:113, wstart:wstart + WIN_COLS],
                )
                return slab

            slab = load_window(0)
            for idx in range(len(windows)):
                # Prefetch the next window before touching this one so its
                # DMAs overlap this window's compute (keeps PE warm too).
                slab_next = (
                    load_window(idx + 1) if idx + 1 < len(windows) else None
                )
                i, wi = windows[idx]
                wstart = wi * WIN_STRIDE
                if True:
                    blk0 = wi * WIN_BLOCKS
                    coff = 0
                    for nb in CHUNKS:
                        mn = stage_pool.tile([128, 8, J], f32, tag="mn")
                        for q in range(0, nb, 2):
                            npair = min(2, nb - q)
                            # 2-bank PSUM tile: sub-block s at elem offset
                            # s*512 (bank-aligned) so one DVE reduce covers
                            # both blocks, halving reduce-op overhead.
                            ps = psum_pool.tile([128, 2, 512], f32)
                            for s in range(npair):
                                b = blk0 + coff + q + s
                                rel = b * BLK - wstart
                                for kh in range(3):
                                    s0 = rel + kh * W
                                    lhsT = (
                                        slab[:, s0:s0 + BLK]
                                        .rearrange("p (m j) -> p m j", m=128)
                                        [:, :, 0:1]
                                    )
                                    nc.tensor.matmul(
                                        ps[:, s, 0:NFREE],
                                        lhsT,
                                        w_t[:, kh * NFREE:(kh + 1) * NFREE],
                                        start=(kh == 0),
                                        stop=(kh == 2),
                                    )
                            nc.vector.tensor_reduce(
                                mn[:, q:q + npair, :],
                                ps[:, 0:npair, 0:NFREE].rearrange(
                                    "p s (j o) -> p s j o", o=C_OUT
                                ),
                                axis=mybir.AxisListType.X,
                                op=mybir.AluOpType.min,
                            )
                        th = stage_pool.tile([128, 8, J], f32, tag="th")
                        nc.scalar.activation(
                            th[:, 0:nb, :], mn[:, 0:nb, :],
                            mybir.ActivationFunctionType.Tanh,
                        )
                        nc.scalar.activation(
                            th[:, 0:nb, :], th[:, 0:nb, :],
                            mybir.ActivationFunctionType.Tanh,
                        )
                        cb = (blk0 + coff) * BLK
                        dst = y_d[i, cb:cb + nb * BLK].rearrange(
                            "(g m j) -> m g j", g=nb, m=128
                        )
                        # SWDGE queue: keeps output stores off the Sync FIFO
                        # so they never delay the slab prefetch DMAs.
                        nc.gpsimd.dma_start(dst, th[:, 0:nb, :])
                        coff += nb
                    slab = slab_next
    nc.compile()
    return nc


_NC_CACHE = []


def _get_nc():
    if not _NC_CACHE:
        _NC_CACHE.append(_build_program())
    return _NC_CACHE[0]


def kernel(x, conv_weight, conv_bias, _trace=False):
    x = np.asarray(x, dtype=np.float32)
    conv_weight = np.asarray(conv_weight, dtype=np.float32)
    conv_bias = np.asarray(conv_bias, dtype=np.float32)
    n = x.shape[0]
    assert n == N_CORES * IMGS_PER_CORE

    x_aug, wmov = _prep_inputs(x, conv_weight, conv_bias)
    nc = _get_nc()
    in_maps = [
        {
            "x": np.ascontiguousarray(
                x_aug[c * IMGS_PER_CORE:(c + 1) * IMGS_PER_CORE]
            ),
            "w": wmov,
        }
        for c in range(N_CORES)
    ]
    res = run_bass_kernel_spmd(
        nc, in_maps, core_ids=list(range(N_CORES)), trace=_trace
    )
    y = np.concatenate([r["y"] for r in res.results], axis=0)  # [32, 65280]
    y = y.reshape(n, 1, 255, 256)[:, :, :OH, :OW]
    out = np.ascontiguousarray(y)
    if _trace:
        kernel._last_result = res
    return out

